# revision 28
# baseline (speedup 1.0000x reference)
"""CFConv (SchNet continuous-filter convolution) kernel for Trainium2, 8 NeuronCores.

Computation (reference):
    f    = x @ W_in2fac                      # (NA, 128)
    f_j  = f[idx_j]                          # (NI, 128) gather
    wf   = w * f_j                           # elementwise
    conv = segment_sum(wf, seg_i, NA)        # (NA, 128), seg_i sorted
    y    = conv @ W_fac2out + b_fac2out      # (NA, 128)

Distribution strategy (graph partition by atom, per the sharding hint):
  * Atoms are sharded contiguously across the 8 cores (12500 atoms each).
    Because seg_i is sorted, each core owns a contiguous slice of the
    interaction list; no cross-core halo is needed for the segment-sum.
  * The small Dense weights are replicated.
  * The gather source table f is replicated: every core computes the full
    f = x @ W_in2fac (cheap) and writes it to its private HBM as fp16,
    then gathers its neighbor rows with dma_gather spread over 4 SWDGE
    queues (4 concurrent Q7 queue workers; each is transfer-rate bound,
    so fp16 rows + 4 queues give ~4.5x the single-queue fp32 rate).

On-core algorithm:
  * f precompute: host supplies x^T (fp16); f-tiles come out of the PE
    row-major and are stored fp16 to 4 chunk tables in HBM (25088 rows
    each) so int16 gather indices stay in range.
  * Interactions are reordered host-side by (j-chunk, atom-block) with each
    group padded to a multiple of 128 (pad entries have w=0). Group tile
    counts are maxed across cores so one SPMD program fits all 8 cores.
  * Per 128-interaction tile: wf = w * f_j (DVE, fp16), selection matrix
    S[p, q] = (segcol[p] == q) built by one DVE tensor_scalar is_equal
    against a constant iota row, then PE computes convT += wf^T_as_lhsT @ S
    accumulating into a 512-atom-wide PSUM bank (block atoms per matmul,
    512//block blocks share the bank; one DVE flush per 512-atom group).
  * fac2out: y_block = convT_block^T @ W_fac2out + bias (bias folded in as
    a K=1 matmul), streamed out per 128 atoms.
"""

import math
import os
import sys

import numpy as np

import concourse.bass as bass
import concourse.mybir as mybir
import concourse.tile as tile
from concourse import bacc
from concourse.bass_utils import run_bass_kernel_spmd

F32 = mybir.dt.float32
F16 = mybir.dt.float16
I16 = mybir.dt.int16
I32 = mybir.dt.int32


class Cfg:
    def __init__(self, na, ni, n_cores, block=256, slab=1024, xslab=2048,
                 crows_list=(26624, 26624, 26624, 26624)):
        self.na = na                    # total atoms
        self.ni = ni                    # total interactions
        self.n_cores = n_cores
        self.apc = na // n_cores        # atoms per core
        self.block = block              # atoms per segment-sum matmul
        self.nb = math.ceil(self.apc / block)   # blocks per core
        self.pg = 512 // block          # blocks per PSUM accumulation group
        self.ng512 = math.ceil(self.nb / self.pg)  # 512-atom groups per core
        # geometric chunk sizes: tiny first chunk so gathers start early,
        # later tables built under the gather stream of earlier chunks
        self.crows_list = list(crows_list)
        self.cbounds = np.concatenate([[0], np.cumsum(self.crows_list)])
        self.nch = len(self.crows_list)
        self.slab = slab                # interactions per pipeline slab
        self.xslab = xslab              # atoms per x^T load slab
        assert 512 % block == 0
        for cr in self.crows_list:
            assert cr % self.xslab == 0 and cr <= 32768
        assert int(self.cbounds[-1]) >= na
        assert na % n_cores == 0
        assert slab <= 1024             # dma_gather num_idxs limit


FULL = dict(na=100_000, ni=1_600_000, n_cores=8)


def _plan(seg, idx_j, cfg):
    """Host-side graph partition + padding plan."""
    nb, nch, block = cfg.nb, cfg.nch, cfg.block
    ngroups = nch * nb
    counts = np.zeros((cfg.n_cores, ngroups), dtype=np.int64)
    per_core_raw = []
    bounds = np.searchsorted(seg, np.arange(cfg.n_cores + 1) * cfg.apc)
    for c in range(cfg.n_cores):
        e0, e1 = bounds[c], bounds[c + 1]
        ls = (seg[e0:e1] - c * cfg.apc).astype(np.int64)
        blk = ls // block
        col = ls - blk * block
        j = idx_j[e0:e1].astype(np.int64)
        jc = np.searchsorted(cfg.cbounds, j, side="right") - 1
        jl = (j - cfg.cbounds[jc]).astype(np.int16)
        key = (jc * nb + blk).astype(np.int64)
        order = np.argsort(key, kind="stable")
        counts[c] = np.bincount(key, minlength=ngroups)
        per_core_raw.append((e0, e1, order, key[order], jl[order], col[order]))

    T = np.ceil(counts.max(axis=0) / 128.0).astype(np.int64)   # tiles per group
    cap = T * 128
    cap_off = np.concatenate([[0], np.cumsum(cap)])
    E_pad = int(cap_off[-1])
    ch_off = [int(cap_off[k * nb]) for k in range(nch)] + [E_pad]

    # per-tile metadata: (chunk, local block, run index, run length)
    meta = []
    for g in range(ngroups):
        for r in range(int(T[g])):
            meta.append((g // nb, g % nb, r, int(T[g])))

    per_core = []
    for c in range(cfg.n_cores):
        e0, e1, order, key_s, jl_s, col_s = per_core_raw[c]
        n = e1 - e0
        data_off = np.concatenate([[0], np.cumsum(counts[c])])[:-1]
        pos = cap_off[key_s] + (np.arange(n) - data_off[key_s])
        per_core.append(dict(e0=int(e0), e1=int(e1), order=order, pos=pos,
                             jl_s=jl_s, col_s=col_s))
    return T, cap_off, E_pad, ch_off, meta, per_core


def _pack_core_inputs(cfg, w, plan_core, E_pad, xT16, w1_16, w2_32, bias_32):
    """Build the per-core in_map (all host-side numpy)."""
    d = 128
    e0, e1, order, pos = (plan_core["e0"], plan_core["e1"],
                          plan_core["order"], plan_core["pos"])
    seg_sorted_col = plan_core["col_s"]
    jl_s = plan_core["jl_s"]

    w_perm = np.zeros((E_pad, d), dtype=np.float16)
    w_perm[pos] = w[e0:e1][order].astype(np.float16)
    ww = np.ascontiguousarray(
        w_perm.reshape(-1, 128, d).transpose(1, 0, 2).reshape(128, -1))

    idx16 = np.zeros(E_pad, dtype=np.int16)
    idx16[pos] = jl_s
    idx_wrap = np.ascontiguousarray(
        np.tile(idx16.reshape(-1, 16).T, (8, 1)))          # [128, E_pad//16]

    segc = np.zeros(E_pad, dtype=np.float16)
    segc[pos] = seg_sorted_col.astype(np.float16)
    seg_wrap = np.ascontiguousarray(segc.reshape(-1, 128).T)  # [128, E_pad//128]

    return {
        "xT": xT16, "w1": w1_16, "w2": w2_32, "bias": bias_32,
        "ww": ww, "idx": idx_wrap, "segcol": seg_wrap,
    }


def _build(cfg, T, ch_off, E_pad, meta):
    """Build + compile the SPMD Bass program (identical for all cores)."""
    from contextlib import ExitStack

    nb, block, nch = cfg.nb, cfg.block, cfg.nch
    pg = cfg.pg
    d = 128
    nc = bacc.Bacc("TRN2", target_bir_lowering=False, debug=False,
                   num_devices=cfg.n_cores, num_swdge_queues=4)

    xT_d = nc.dram_tensor("xT", [d, int(cfg.cbounds[-1])], F16, kind="ExternalInput")
    w1_d = nc.dram_tensor("w1", [d, d], F16, kind="ExternalInput")
    w2_d = nc.dram_tensor("w2", [d, d], F32, kind="ExternalInput")
    bias_d = nc.dram_tensor("bias", [1, d], F32, kind="ExternalInput")
    ww_d = nc.dram_tensor("ww", [d, E_pad], F16, kind="ExternalInput")
    idx_d = nc.dram_tensor("idx", [d, E_pad // 16], I16, kind="ExternalInput")
    seg_d = nc.dram_tensor("segcol", [d, E_pad // 128], F16, kind="ExternalInput")
    f_d = [nc.dram_tensor(f"ftab{k}", [cfg.crows_list[k], d], F16)
           for k in range(nch)]
    y_d = nc.dram_tensor("y", [cfg.apc, d], F32, kind="ExternalOutput")

    with tile.TileContext(nc) as tc, ExitStack() as ctx:
        cpool = ctx.enter_context(tc.tile_pool(name="const", bufs=1))
        xpool = ctx.enter_context(tc.tile_pool(name="xt", bufs=2))
        fps = ctx.enter_context(tc.tile_pool(name="fps", bufs=2, space="PSUM"))
        fst = ctx.enter_context(tc.tile_pool(name="fst", bufs=3))
        idxp = ctx.enter_context(tc.tile_pool(name="idxp", bufs=4))
        gp = ctx.enter_context(tc.tile_pool(name="gp", bufs=20))
        wp = ctx.enter_context(tc.tile_pool(name="wp", bufs=12))
        wfp = ctx.enter_context(tc.tile_pool(name="wfp", bufs=8))
        sp = ctx.enter_context(tc.tile_pool(name="sp", bufs=4))
        sps = ctx.enter_context(tc.tile_pool(name="sps", bufs=5, space="PSUM"))
        yps = ctx.enter_context(tc.tile_pool(name="yps", bufs=1, space="PSUM"))
        yst = ctx.enter_context(tc.tile_pool(name="yst", bufs=3))

        # ---- constants ----
        iota_i = cpool.tile([d, block], I32)
        nc.gpsimd.iota(iota_i[:], pattern=[[1, block]], base=0,
                       channel_multiplier=0)
        iota_h = cpool.tile([d, block], F16)
        nc.vector.tensor_copy(iota_h[:], iota_i[:])
        w1_t = cpool.tile([d, d], F16)
        nc.sync.dma_start(out=w1_t[:], in_=w1_d[:, :])
        w2_t = cpool.tile([d, d], F32)
        nc.sync.dma_start(out=w2_t[:], in_=w2_d[:, :])
        bias_t = cpool.tile([1, d], F32)
        nc.sync.dma_start(out=bias_t[:], in_=bias_d[:, :])
        ones_t = cpool.tile([1, d], F32)
        nc.vector.memset(ones_t[:], 1.0)
        seg_t = cpool.tile([d, E_pad // 128], F16)
        nc.sync.dma_start(out=seg_t[:], in_=seg_d[:, :])
        convT = cpool.tile([d, cfg.ng512 * 512], F32)
        nc.vector.memset(convT[:], 0.0)

        # 512-atom PSUM accumulation group shared by pg consecutive blocks;
        # flushed with one DVE add when the group retires.
        grp_state = {"gid": None, "tile": None, "dirty": False}

        def flush_grp():
            pt = grp_state["tile"]
            if pt is None or not grp_state["dirty"]:
                grp_state["gid"] = None
                grp_state["tile"] = None
                return
            g = grp_state["gid"][1]            # 512-atom group index
            c0 = g * 512
            nc.vector.tensor_add(convT[:, c0:c0 + 512],
                                 convT[:, c0:c0 + 512], pt[:])
            grp_state["gid"] = None
            grp_state["tile"] = None
            grp_state["dirty"] = False

        qrr = [0]  # SWDGE queue round-robin counter
        cleared = [0]  # next 512-atom group to emit fac2out for

        def emit_C(g):
            a_lo = g * 512
            a_hi = min(a_lo + 512, cfg.apc)
            for a0 in range(a_lo, a_hi, 128):
                m = min(128, a_hi - a0)
                yp = yps.tile([d, d], F32)
                nc.tensor.matmul(out=yp[:m, :], lhsT=convT[:, a0:a0 + m],
                                 rhs=w2_t[:], start=True, stop=False)
                nc.tensor.matmul(out=yp[:m, :], lhsT=ones_t[:, :m],
                                 rhs=bias_t[:], start=False, stop=True)
                ys = yst.tile([d, d], F32)
                nc.scalar.copy(ys[:m, :], yp[:m, :])
                nc.scalar.dma_start(out=y_d[a0:a0 + m, :], in_=ys[:m, :])

        for k in range(nch):
            # ---- phase A(k): f16 f table for chunk k ----
            base_col = int(cfg.cbounds[k])
            crows_k = cfg.crows_list[k]
            for off in range(0, crows_k, cfg.xslab):
                sz = min(cfg.xslab, crows_k - off)
                xt = xpool.tile([d, cfg.xslab], F16)
                nc.scalar.dma_start(out=xt[:, :sz],
                                    in_=xT_d[:, base_col + off: base_col + off + sz])
                for g in range(sz // 512):
                    ps = fps.tile([d, 512], F32)
                    for i in range(4):
                        nc.tensor.matmul(
                            out=ps[:, i * 128:(i + 1) * 128],
                            lhsT=xt[:, g * 512 + i * 128: g * 512 + (i + 1) * 128],
                            rhs=w1_t[:], start=True, stop=True)
                    st = fst.tile([d, 512], F16)
                    nc.scalar.copy(st[:], ps[:])
                    row0 = off + g * 512
                    nc.scalar.dma_start(
                        out=f_d[k][row0:row0 + 512, :].rearrange(
                            "(a p) c -> p a c", p=128),
                        in_=st[:].rearrange("p (a c) -> p a c", a=4))

        ISS = 8 * cfg.slab              # idx superslab (8 slabs per load)
        for k in range(nch):
            # ---- phase B(k): gather + filter + segment-sum ----
            idx_state = {"tile": None, "s0": -1}
            for s in range(ch_off[k], ch_off[k + 1], cfg.slab):
                L = min(cfg.slab, ch_off[k + 1] - s)
                nt = L // 128
                s0 = ch_off[k] + ((s - ch_off[k]) // ISS) * ISS
                if s0 != idx_state["s0"]:
                    iL = min(ISS, ch_off[k + 1] - s0)
                    it = idxp.tile([d, ISS // 16], I16)
                    nc.sync.dma_start(out=it[:, :iL // 16],
                                      in_=idx_d[:, s0 // 16:(s0 + iL) // 16])
                    idx_state = {"tile": it, "s0": s0}
                io0 = (s - s0) // 16
                idxt = idx_state["tile"]
                gt = gp.tile([d, cfg.slab], F16)
                nc.gpsimd.dma_gather(
                    gt[:, :L].rearrange("p (n c) -> p n c", c=128),
                    f_d[k][:, :],
                    idxt[:, io0:io0 + L // 16],
                    L, L, 128, elem_step=128, queue_num=qrr[0] % 4,
                    single_packet=False)
                qrr[0] += 1
                wt = wp.tile([d, cfg.slab], F16)
                nc.sync.dma_start(out=wt[:, :L], in_=ww_d[:, s:s + L])
                wft = wfp.tile([d, cfg.slab], F16)
                nc.vector.tensor_mul(wft[:, :L], wt[:, :L], gt[:, :L])
                # one broadcast is_equal builds all nt selection matrices
                t0 = s // 128
                S8 = sp.tile([d, (cfg.slab // 128) * block], F16)
                io = iota_h[:, :block]
                iob = bass.AP(io.tensor, io.offset,
                              [list(io.ap[0]), [0, nt], list(io.ap[1])])
                nc.vector.tensor_tensor(
                    out=S8[:, :nt * block].rearrange("p (t q) -> p t q",
                                                     q=block),
                    in0=seg_t[:, t0:t0 + nt].to_broadcast([d, nt, block]),
                    in1=iob, op=mybir.AluOpType.is_equal)
                for lt in range(nt):
                    t = t0 + lt
                    _k2, b, r, Tg = meta[t]
                    gid = (k, b // pg)
                    if gid != grp_state["gid"]:
                        flush_grp()
                        if k == nch - 1:
                            while cleared[0] < b // pg:
                                emit_C(cleared[0])
                                cleared[0] += 1
                        grp_state["gid"] = gid
                        grp_state["tile"] = sps.tile(
                            [d, 512], F32, name="grp_ps", tag="grp_ps")
                    sub = b % pg
                    pt = grp_state["tile"]
                    nc.tensor.matmul(out=pt[:, sub * block:(sub + 1) * block],
                                     lhsT=wft[:, lt * 128:(lt + 1) * 128],
                                     rhs=S8[:, lt * block:(lt + 1) * block],
                                     start=(r == 0), stop=(r == Tg - 1))
                    if r == Tg - 1:
                        grp_state["dirty"] = True

        flush_grp()
        while cleared[0] < cfg.ng512:
            emit_C(cleared[0])
            cleared[0] += 1

    nc.compile()
    return nc


def _choose_block(seg, idx_j, cfg_base):
    """Pick the atom-block size with the best engine-cost proxy."""
    best = None
    for block in (128, 256, 512):
        cfg = Cfg(**FULL)
        cfg.block = block
        cfg.nb = math.ceil(cfg.apc / block)
        cfg.pg = 512 // block
        cfg.ng512 = math.ceil(cfg.nb / cfg.pg)
        T, _, E_pad, _, _, _ = _plan(seg, idx_j, cfg)
        tiles = E_pad // 128
        slabs = E_pad / 1024
        dve = slabs * (8 * block * 0.54 + 900) + 100 * 700
        pe = tiles * (block * 0.73 + 40) + 110000
        gather = E_pad * 2.2
        cost = max(dve, pe, gather)
        print(f"  block={block}: E_pad={E_pad} tiles={tiles} "
              f"dve={dve/1000:.0f}us pe={pe/1000:.0f}us "
              f"gather={gather/1000:.0f}us", file=sys.stderr)
        if best is None or cost < best[0]:
            best = (cost, block, E_pad)
    return best[1]


def _run(inputs, cfg=None, trace=False, tmpdir=None):
    d = 128

    x = np.asarray(inputs["x"], dtype=np.float32)
    w = np.asarray(inputs["w"], dtype=np.float32)
    seg = np.asarray(inputs["seg_i"]).astype(np.int64)
    idx_j = np.asarray(inputs["idx_j"]).astype(np.int64)
    W1 = np.asarray(inputs["W_in2fac"], dtype=np.float32)
    W2 = np.asarray(inputs["W_fac2out"], dtype=np.float32)
    b = np.asarray(inputs["b_fac2out"], dtype=np.float32)

    if cfg is None:
        cfg = Cfg(**FULL, block=256)

    T, cap_off, E_pad, ch_off, meta, per_core = _plan(seg, idx_j, cfg)

    xT16 = np.zeros((d, int(cfg.cbounds[-1])), dtype=np.float16)
    xT16[:, :cfg.na] = x.T.astype(np.float16)
    w1_16 = np.ascontiguousarray(W1.astype(np.float16))
    w2_32 = np.ascontiguousarray(W2)
    bias_32 = np.ascontiguousarray(b[None, :])

    in_maps = []
    for c in range(cfg.n_cores):
        in_maps.append(_pack_core_inputs(cfg, w, per_core[c], E_pad, xT16,
                                         w1_16, w2_32, bias_32))

    nc = _build(cfg, T, ch_off, E_pad, meta)

    res = run_bass_kernel_spmd(nc, in_maps, core_ids=list(range(cfg.n_cores)),
                               tmpdir=tmpdir, trace=trace)
    y = np.concatenate([res.results[c]["y"] for c in range(cfg.n_cores)], axis=0)
    return y[:cfg.na], res, nc, in_maps


def kernel(**inputs) -> np.ndarray:
    y, _res, _nc, _maps = _run(inputs)
    return y


# revision 30
# speedup vs baseline: 1.2004x; 1.2004x over previous
"""CFConv (SchNet continuous-filter convolution) kernel for Trainium2, 8 NeuronCores.

Computation (reference):
    f    = x @ W_in2fac                      # (NA, 128)
    f_j  = f[idx_j]                          # (NI, 128) gather
    wf   = w * f_j                           # elementwise
    conv = segment_sum(wf, seg_i, NA)        # (NA, 128), seg_i sorted
    y    = conv @ W_fac2out + b_fac2out      # (NA, 128)

Distribution strategy (graph partition by atom, per the sharding hint):
  * Atoms are sharded contiguously across the 8 cores (12500 atoms each).
    Because seg_i is sorted, each core owns a contiguous slice of the
    interaction list; no cross-core halo is needed for the segment-sum.
  * The small Dense weights are replicated.
  * The gather source table f is replicated: every core computes the full
    f = x @ W_in2fac (cheap) and writes it to its private HBM as fp16,
    then gathers its neighbor rows with dma_gather spread over 4 SWDGE
    queues (4 concurrent Q7 queue workers; each is transfer-rate bound,
    so fp16 rows + 4 queues give ~4.5x the single-queue fp32 rate).

On-core algorithm:
  * f precompute: host supplies x^T (fp16); f-tiles come out of the PE
    row-major and are stored fp16 to 4 chunk tables in HBM (25088 rows
    each) so int16 gather indices stay in range.
  * Interactions are reordered host-side by (j-chunk, atom-block) with each
    group padded to a multiple of 128 (pad entries have w=0). Group tile
    counts are maxed across cores so one SPMD program fits all 8 cores.
  * Per 128-interaction tile: wf = w * f_j (DVE, fp16), selection matrix
    S[p, q] = (segcol[p] == q) built by one DVE tensor_scalar is_equal
    against a constant iota row, then PE computes convT += wf^T_as_lhsT @ S
    accumulating into a 512-atom-wide PSUM bank (block atoms per matmul,
    512//block blocks share the bank; one DVE flush per 512-atom group).
  * fac2out: y_block = convT_block^T @ W_fac2out + bias (bias folded in as
    a K=1 matmul), streamed out per 128 atoms.
"""

import math
import os
import sys

import numpy as np

import concourse.bass as bass
import concourse.mybir as mybir
import concourse.tile as tile
from concourse import bacc
from concourse.bass_utils import run_bass_kernel_spmd

F32 = mybir.dt.float32
F16 = mybir.dt.float16
I16 = mybir.dt.int16
I32 = mybir.dt.int32


class Cfg:
    def __init__(self, na, ni, n_cores, block=256, slab=1024, xslab=2048,
                 crows_list=(26624, 26624, 26624, 26624)):
        self.na = na                    # total atoms
        self.ni = ni                    # total interactions
        self.n_cores = n_cores
        self.apc = na // n_cores        # atoms per core
        self.block = block              # atoms per segment-sum matmul
        self.nb = math.ceil(self.apc / block)   # blocks per core
        self.pg = 512 // block          # blocks per PSUM accumulation group
        self.ng512 = math.ceil(self.nb / self.pg)  # 512-atom groups per core
        # geometric chunk sizes: tiny first chunk so gathers start early,
        # later tables built under the gather stream of earlier chunks
        self.crows_list = list(crows_list)
        self.cbounds = np.concatenate([[0], np.cumsum(self.crows_list)])
        self.nch = len(self.crows_list)
        self.slab = slab                # interactions per pipeline slab
        self.xslab = xslab              # atoms per x^T load slab
        assert 512 % block == 0
        for cr in self.crows_list:
            assert cr % self.xslab == 0 and cr <= 32768
        assert int(self.cbounds[-1]) >= na
        assert na % n_cores == 0
        assert slab <= 1024             # dma_gather num_idxs limit


FULL = dict(na=100_000, ni=1_600_000, n_cores=8)


def _plan(seg, idx_j, cfg):
    """Host-side graph partition + padding plan."""
    nb, nch, block = cfg.nb, cfg.nch, cfg.block
    ngroups = nch * nb
    counts = np.zeros((cfg.n_cores, ngroups), dtype=np.int64)
    per_core_raw = []
    bounds = np.searchsorted(seg, np.arange(cfg.n_cores + 1) * cfg.apc)
    for c in range(cfg.n_cores):
        e0, e1 = bounds[c], bounds[c + 1]
        ls = (seg[e0:e1] - c * cfg.apc).astype(np.int64)
        blk = ls // block
        col = ls - blk * block
        j = idx_j[e0:e1].astype(np.int64)
        jc = np.searchsorted(cfg.cbounds, j, side="right") - 1
        jl = (j - cfg.cbounds[jc]).astype(np.int16)
        key = (jc * nb + blk).astype(np.int64)
        order = np.argsort(key, kind="stable")
        counts[c] = np.bincount(key, minlength=ngroups)
        per_core_raw.append((e0, e1, order, key[order], jl[order], col[order]))

    T = np.ceil(counts.max(axis=0) / 128.0).astype(np.int64)   # tiles per group
    cap = T * 128
    cap_off = np.concatenate([[0], np.cumsum(cap)])
    E_pad = int(cap_off[-1])
    ch_off = [int(cap_off[k * nb]) for k in range(nch)] + [E_pad]

    # per-tile metadata: (chunk, local block, run index, run length)
    meta = []
    for g in range(ngroups):
        for r in range(int(T[g])):
            meta.append((g // nb, g % nb, r, int(T[g])))

    per_core = []
    for c in range(cfg.n_cores):
        e0, e1, order, key_s, jl_s, col_s = per_core_raw[c]
        n = e1 - e0
        data_off = np.concatenate([[0], np.cumsum(counts[c])])[:-1]
        pos = cap_off[key_s] + (np.arange(n) - data_off[key_s])
        per_core.append(dict(e0=int(e0), e1=int(e1), order=order, pos=pos,
                             jl_s=jl_s, col_s=col_s))
    return T, cap_off, E_pad, ch_off, meta, per_core


def _pack_core_inputs(cfg, w, plan_core, E_pad, xT16, w1_16, w2_32, bias_32):
    """Build the per-core in_map (all host-side numpy)."""
    d = 128
    e0, e1, order, pos = (plan_core["e0"], plan_core["e1"],
                          plan_core["order"], plan_core["pos"])
    seg_sorted_col = plan_core["col_s"]
    jl_s = plan_core["jl_s"]

    w_perm = np.zeros((E_pad, d), dtype=np.float16)
    w_perm[pos] = w[e0:e1][order].astype(np.float16)
    ww = np.ascontiguousarray(
        w_perm.reshape(-1, 128, d).transpose(1, 0, 2).reshape(128, -1))

    idx16 = np.zeros(E_pad, dtype=np.int16)
    idx16[pos] = jl_s
    idx_wrap = np.ascontiguousarray(
        np.tile(idx16.reshape(-1, 16).T, (8, 1)))          # [128, E_pad//16]

    segc = np.zeros(E_pad, dtype=np.float16)
    segc[pos] = seg_sorted_col.astype(np.float16)
    seg_wrap = np.ascontiguousarray(segc.reshape(-1, 128).T)  # [128, E_pad//128]

    return {
        "xT": xT16, "w1": w1_16, "w2": w2_32, "bias": bias_32,
        "ww": ww, "idx": idx_wrap, "segcol": seg_wrap,
    }


def _build(cfg, T, ch_off, E_pad, meta):
    """Build + compile the SPMD Bass program (identical for all cores)."""
    from contextlib import ExitStack

    nb, block, nch = cfg.nb, cfg.block, cfg.nch
    pg = cfg.pg
    d = 128
    nc = bacc.Bacc("TRN2", target_bir_lowering=False, debug=False,
                   num_devices=cfg.n_cores, num_swdge_queues=4)

    xT_d = nc.dram_tensor("xT", [d, int(cfg.cbounds[-1])], F16, kind="ExternalInput")
    w1_d = nc.dram_tensor("w1", [d, d], F16, kind="ExternalInput")
    w2_d = nc.dram_tensor("w2", [d, d], F32, kind="ExternalInput")
    bias_d = nc.dram_tensor("bias", [1, d], F32, kind="ExternalInput")
    ww_d = nc.dram_tensor("ww", [d, E_pad], F16, kind="ExternalInput")
    idx_d = nc.dram_tensor("idx", [d, E_pad // 16], I16, kind="ExternalInput")
    seg_d = nc.dram_tensor("segcol", [d, E_pad // 128], F16, kind="ExternalInput")
    f_d = [nc.dram_tensor(f"ftab{k}", [cfg.crows_list[k], d], F16)
           for k in range(nch)]
    y_d = nc.dram_tensor("y", [cfg.apc, d], F32, kind="ExternalOutput")

    with tile.TileContext(nc) as tc, ExitStack() as ctx:
        cpool = ctx.enter_context(tc.tile_pool(name="const", bufs=1))
        xpool = ctx.enter_context(tc.tile_pool(name="xt", bufs=2))
        fps = ctx.enter_context(tc.tile_pool(name="fps", bufs=2, space="PSUM"))
        fst = ctx.enter_context(tc.tile_pool(name="fst", bufs=3))
        idxp = ctx.enter_context(tc.tile_pool(name="idxp", bufs=4))
        gp = ctx.enter_context(tc.tile_pool(name="gp", bufs=20))
        wp = ctx.enter_context(tc.tile_pool(name="wp", bufs=12))
        wfp = ctx.enter_context(tc.tile_pool(name="wfp", bufs=8))
        sp = ctx.enter_context(tc.tile_pool(name="sp", bufs=4))
        sps = ctx.enter_context(tc.tile_pool(name="sps", bufs=5, space="PSUM"))
        yps = ctx.enter_context(tc.tile_pool(name="yps", bufs=1, space="PSUM"))
        yst = ctx.enter_context(tc.tile_pool(name="yst", bufs=3))

        # ---- constants ----
        iota_i = cpool.tile([d, block], I32)
        nc.gpsimd.iota(iota_i[:], pattern=[[1, block]], base=0,
                       channel_multiplier=0)
        iota_h = cpool.tile([d, block], F16)
        nc.vector.tensor_copy(iota_h[:], iota_i[:])
        w1_t = cpool.tile([d, d], F16)
        nc.sync.dma_start(out=w1_t[:], in_=w1_d[:, :])
        w2_t = cpool.tile([d, d], F32)
        nc.sync.dma_start(out=w2_t[:], in_=w2_d[:, :])
        bias_t = cpool.tile([1, d], F32)
        nc.sync.dma_start(out=bias_t[:], in_=bias_d[:, :])
        ones_t = cpool.tile([1, d], F32)
        nc.vector.memset(ones_t[:], 1.0)
        seg_t = cpool.tile([d, E_pad // 128], F16)
        nc.sync.dma_start(out=seg_t[:], in_=seg_d[:, :])
        convT = cpool.tile([d, cfg.ng512 * 512], F32)
        nc.vector.memset(convT[:], 0.0)

        # 512-atom PSUM accumulation group shared by pg consecutive blocks;
        # flushed with one DVE add when the group retires.
        grp_state = {"gid": None, "tile": None, "dirty": False}

        def flush_grp():
            pt = grp_state["tile"]
            if pt is None or not grp_state["dirty"]:
                grp_state["gid"] = None
                grp_state["tile"] = None
                return
            g = grp_state["gid"][1]            # 512-atom group index
            c0 = g * 512
            nc.vector.tensor_add(convT[:, c0:c0 + 512],
                                 convT[:, c0:c0 + 512], pt[:])
            grp_state["gid"] = None
            grp_state["tile"] = None
            grp_state["dirty"] = False

        qrr = [0]  # SWDGE queue round-robin counter
        cleared = [0]  # next 512-atom group to emit fac2out for

        def emit_C(g):
            a_lo = g * 512
            a_hi = min(a_lo + 512, cfg.apc)
            for a0 in range(a_lo, a_hi, 128):
                m = min(128, a_hi - a0)
                yp = yps.tile([d, d], F32)
                nc.tensor.matmul(out=yp[:m, :], lhsT=convT[:, a0:a0 + m],
                                 rhs=w2_t[:], start=True, stop=False)
                nc.tensor.matmul(out=yp[:m, :], lhsT=ones_t[:, :m],
                                 rhs=bias_t[:], start=False, stop=True)
                ys = yst.tile([d, d], F32)
                nc.scalar.copy(ys[:m, :], yp[:m, :])
                nc.scalar.dma_start(out=y_d[a0:a0 + m, :], in_=ys[:m, :])

        for k in range(nch):
            # ---- phase A(k): f16 f table for chunk k ----
            base_col = int(cfg.cbounds[k])
            crows_k = cfg.crows_list[k]
            for off in range(0, crows_k, cfg.xslab):
                sz = min(cfg.xslab, crows_k - off)
                xt = xpool.tile([d, cfg.xslab], F16)
                nc.scalar.dma_start(out=xt[:, :sz],
                                    in_=xT_d[:, base_col + off: base_col + off + sz])
                for g in range(sz // 512):
                    ps = fps.tile([d, 512], F32)
                    for i in range(4):
                        nc.tensor.matmul(
                            out=ps[:, i * 128:(i + 1) * 128],
                            lhsT=xt[:, g * 512 + i * 128: g * 512 + (i + 1) * 128],
                            rhs=w1_t[:], start=True, stop=True)
                    st = fst.tile([d, 512], F16)
                    nc.scalar.copy(st[:], ps[:])
                    row0 = off + g * 512
                    nc.scalar.dma_start(
                        out=f_d[k][row0:row0 + 512, :].rearrange(
                            "(a p) c -> p a c", p=128),
                        in_=st[:].rearrange("p (a c) -> p a c", a=4))

        ISS = 32 * cfg.slab             # idx superslab (32 slabs per load)
        for k in range(nch):
            # ---- phase B(k): gather + filter + segment-sum ----
            idx_state = {"tile": None, "s0": -1}
            for s in range(ch_off[k], ch_off[k + 1], cfg.slab):
                L = min(cfg.slab, ch_off[k + 1] - s)
                nt = L // 128
                s0 = ch_off[k] + ((s - ch_off[k]) // ISS) * ISS
                if s0 != idx_state["s0"]:
                    iL = min(ISS, ch_off[k + 1] - s0)
                    it = idxp.tile([d, ISS // 16], I16)
                    nc.sync.dma_start(out=it[:, :iL // 16],
                                      in_=idx_d[:, s0 // 16:(s0 + iL) // 16])
                    idx_state = {"tile": it, "s0": s0}
                io0 = (s - s0) // 16
                idxt = idx_state["tile"]
                gt = gp.tile([d, cfg.slab], F16)
                nc.gpsimd.dma_gather(
                    gt[:, :L].rearrange("p (n c) -> p n c", c=128),
                    f_d[k][:, :],
                    idxt[:, io0:io0 + L // 16],
                    L, L, 128, elem_step=128, queue_num=qrr[0] % 4)
                qrr[0] += 1
                wt = wp.tile([d, cfg.slab], F16)
                nc.sync.dma_start(out=wt[:, :L], in_=ww_d[:, s:s + L])
                wft = wfp.tile([d, cfg.slab], F16)
                nc.vector.tensor_mul(wft[:, :L], wt[:, :L], gt[:, :L])
                # one broadcast is_equal builds all nt selection matrices
                t0 = s // 128
                S8 = sp.tile([d, (cfg.slab // 128) * block], F16)
                io = iota_h[:, :block]
                iob = bass.AP(io.tensor, io.offset,
                              [list(io.ap[0]), [0, nt], list(io.ap[1])])
                nc.vector.tensor_tensor(
                    out=S8[:, :nt * block].rearrange("p (t q) -> p t q",
                                                     q=block),
                    in0=seg_t[:, t0:t0 + nt].to_broadcast([d, nt, block]),
                    in1=iob, op=mybir.AluOpType.is_equal)
                for lt in range(nt):
                    t = t0 + lt
                    _k2, b, r, Tg = meta[t]
                    gid = (k, b // pg)
                    if gid != grp_state["gid"]:
                        flush_grp()
                        if k == nch - 1:
                            while cleared[0] < b // pg:
                                emit_C(cleared[0])
                                cleared[0] += 1
                        grp_state["gid"] = gid
                        grp_state["tile"] = sps.tile(
                            [d, 512], F32, name="grp_ps", tag="grp_ps")
                    sub = b % pg
                    pt = grp_state["tile"]
                    nc.tensor.matmul(out=pt[:, sub * block:(sub + 1) * block],
                                     lhsT=wft[:, lt * 128:(lt + 1) * 128],
                                     rhs=S8[:, lt * block:(lt + 1) * block],
                                     start=(r == 0), stop=(r == Tg - 1))
                    if r == Tg - 1:
                        grp_state["dirty"] = True

        flush_grp()
        while cleared[0] < cfg.ng512:
            emit_C(cleared[0])
            cleared[0] += 1

    nc.compile()
    return nc


def _choose_block(seg, idx_j, cfg_base):
    """Pick the atom-block size with the best engine-cost proxy."""
    best = None
    for block in (128, 256, 512):
        cfg = Cfg(**FULL)
        cfg.block = block
        cfg.nb = math.ceil(cfg.apc / block)
        cfg.pg = 512 // block
        cfg.ng512 = math.ceil(cfg.nb / cfg.pg)
        T, _, E_pad, _, _, _ = _plan(seg, idx_j, cfg)
        tiles = E_pad // 128
        slabs = E_pad / 1024
        dve = slabs * (8 * block * 0.54 + 900) + 100 * 700
        pe = tiles * (block * 0.73 + 40) + 110000
        gather = E_pad * 2.2
        cost = max(dve, pe, gather)
        print(f"  block={block}: E_pad={E_pad} tiles={tiles} "
              f"dve={dve/1000:.0f}us pe={pe/1000:.0f}us "
              f"gather={gather/1000:.0f}us", file=sys.stderr)
        if best is None or cost < best[0]:
            best = (cost, block, E_pad)
    return best[1]


def _run(inputs, cfg=None, trace=False, tmpdir=None):
    d = 128

    x = np.asarray(inputs["x"], dtype=np.float32)
    w = np.asarray(inputs["w"], dtype=np.float32)
    seg = np.asarray(inputs["seg_i"]).astype(np.int64)
    idx_j = np.asarray(inputs["idx_j"]).astype(np.int64)
    W1 = np.asarray(inputs["W_in2fac"], dtype=np.float32)
    W2 = np.asarray(inputs["W_fac2out"], dtype=np.float32)
    b = np.asarray(inputs["b_fac2out"], dtype=np.float32)

    if cfg is None:
        cfg = Cfg(**FULL, block=256)

    T, cap_off, E_pad, ch_off, meta, per_core = _plan(seg, idx_j, cfg)

    xT16 = np.zeros((d, int(cfg.cbounds[-1])), dtype=np.float16)
    xT16[:, :cfg.na] = x.T.astype(np.float16)
    w1_16 = np.ascontiguousarray(W1.astype(np.float16))
    w2_32 = np.ascontiguousarray(W2)
    bias_32 = np.ascontiguousarray(b[None, :])

    in_maps = []
    for c in range(cfg.n_cores):
        in_maps.append(_pack_core_inputs(cfg, w, per_core[c], E_pad, xT16,
                                         w1_16, w2_32, bias_32))

    nc = _build(cfg, T, ch_off, E_pad, meta)

    res = run_bass_kernel_spmd(nc, in_maps, core_ids=list(range(cfg.n_cores)),
                               tmpdir=tmpdir, trace=trace)
    y = np.concatenate([res.results[c]["y"] for c in range(cfg.n_cores)], axis=0)
    return y[:cfg.na], res, nc, in_maps


def kernel(**inputs) -> np.ndarray:
    y, _res, _nc, _maps = _run(inputs)
    return y
